# revision 14
# baseline (speedup 1.0000x reference)
"""Causal self-attention Bass/Tile kernel for Trainium2, 8 NeuronCores.

Problem: B=4, T=2048, C=1024, NH=16, HD=64.
  q/k/v = x @ W{q,k,v}; att = softmax(causal(q k^T / 8)); y = (att v) @ Wp

Sharding (8 cores): batch (4-way) x head-group (2-way tensor parallel).
Core c handles batch b=c//2 and global heads g*8..g*8+7 where g=c%2.
Each core computes a partial projection y_part = y_heads_local @ Wp[rows]
and the host unshards by summing the two partial outputs per batch.

Per-core kernel (all T=2048 tokens, 8 heads, head_dim 64), bf16 matmuls
with fp32 PSUM accumulation and fp32 softmax:
  Phase A: x^T, Wq, Wk, Wv resident in SBUF (bf16); qT/kT = (x W)^T
           stored [d, t], v stored [t, d] augmented with a ones column so
           P^T@[V|1] also yields the softmax denominator l in PSUM row 64.
  Phase B: per query tile j / head pair: transposed score tiles
           S^T [s:128, t:512] on PE with the two heads of the pair on
           disjoint PE row halves (concurrent sub-array execution),
           exp(S/8) on ACT (PSUM->SBUF bf16), causal mask via GPSIMD
           affine_select (fill 0 post-exp), P^T@[V|1] accumulating
           unnormalized out^T [65, t] per head in PSUM.
  Phase C: l -> 1/l (DVE reciprocal), pair-broadcast over 128 head dims
           via a K=2 fp32 selector matmul, normalize out^T (DVE multiply).
  Phase D: y_part[t, c] = sum_u ylocT[u, t] * Wp[u, c] on PE (bf16).
"""

import numpy as np

B, T, C, NH, HD = 4, 2048, 1024, 16, 64
G = 512          # local head dims per core (8 heads x 64)
P = 128
NT = 4           # t tiles of 512
NT128 = 16       # t tiles of 128
NPAIR = 4        # local head pairs
TT = 512

_CACHE = {}


def _build_nc():
    import concourse.tile as tile
    from concourse import bacc, mybir

    f32 = mybir.dt.float32
    bf16 = mybir.dt.bfloat16

    nc = bacc.Bacc("TRN2", target_bir_lowering=False, debug=False)

    xT = nc.dram_tensor("xt", [C, T], bf16, kind="ExternalInput")
    wq = nc.dram_tensor("wq", [C, G], bf16, kind="ExternalInput")
    wk = nc.dram_tensor("wk", [C, G], bf16, kind="ExternalInput")
    wv = nc.dram_tensor("wv", [C, G], bf16, kind="ExternalInput")
    wp = nc.dram_tensor("wp", [G, C], bf16, kind="ExternalInput")
    sel = nc.dram_tensor("sel", [2, P], f32, kind="ExternalInput")
    y = nc.dram_tensor("y", [T, C], f32, kind="ExternalOutput")

    xT_v = xT.rearrange("(co p) t -> p co t", p=P)      # [128, 8, 2048]
    wq_v = wq.rearrange("(co p) g -> p co g", p=P)      # [128, 8, 512]
    wk_v = wk.rearrange("(co p) g -> p co g", p=P)
    wv_v = wv.rearrange("(co p) g -> p co g", p=P)
    wp_v = wp.rearrange("(uo p) c -> p uo c", p=P)      # [128, 4, 1024]
    y_v = y.rearrange("(to p) c -> p to c", p=P)        # [128, 16, 1024]

    with tile.TileContext(nc) as tc:
        with (
            tc.tile_pool(name="singles", bufs=1) as singles,
            tc.tile_pool(name="expst", bufs=1) as epool,
            tc.tile_pool(name="bcast", bufs=1) as bpool,
            tc.tile_pool(name="rf", bufs=2) as rfpool,
            tc.tile_pool(name="ystage", bufs=3) as ypool,
            tc.tile_pool(name="psA", bufs=4, space="PSUM") as psA,
            tc.tile_pool(name="psS", bufs=2, space="PSUM") as psS,
        ):
            # persistent tensors
            xT_sb = singles.tile([P, 8, T], bf16, name="xT_sb", tag="xT_sb")
            # wqk_sb[:, co, 2*dg+view, :]: lhsT tiles for q (view 0), k (view 1)
            wqk_sb = singles.tile([P, 8, 8, P], bf16, name="wqk_sb", tag="wqk_sb")
            wv_sb = singles.tile([P, 8, G], bf16, name="wv_sb", tag="wv_sb")
            wp_sb = singles.tile([P, NPAIR, C], bf16, name="wp_sb", tag="wp_sb")
            qT = singles.tile([P, NPAIR, T], bf16, name="qT", tag="qT")
            kT = singles.tile([P, NPAIR, T], bf16, name="kT", tag="kT")
            v_sb = singles.tile([P, NT128, 8, 66], bf16, name="v_sb", tag="v_sb")
            ylocT = singles.tile([P, NPAIR, T], bf16, name="ylocT", tag="ylocT")
            # l for (h, j) lives at partition 32*j, free slot h (DVE copies
            # out of PSUM row 64 may only target partitions 0/32/64/96);
            # l8 holds pair pr at partitions {32pr, 32pr+1}
            lq = singles.tile([P, 8, TT], f32, name="lq", tag="lq")
            l8 = singles.tile([P, NT, TT], f32, name="l8", tag="l8")
            sel_sb = singles.tile([2, P], f32, name="sel_sb", tag="sel_sb")

            nc.vector.memset(v_sb[:, :, :, 64:65], 1.0)
            nc.vector.memset(l8[:], 1.0)
            nc.sync.dma_start(sel_sb[:], sel[:])
            # load order: dg0 weights first so PE starts ~immediately, then
            # x chunks; remaining weights ride other engines' DMA queues.
            for view, w_view in ((0, wq_v), (1, wk_v)):
                nc.sync.dma_start(
                    wqk_sb[:, :, view, :], w_view[:, :, 0:P])
            for co in range(8):
                nc.sync.dma_start(xT_sb[:, co, :], xT_v[:, co, :])
            for dg in range(1, NPAIR):
                for view, w_view in ((0, wq_v), (1, wk_v)):
                    nc.scalar.dma_start(
                        wqk_sb[:, :, 2 * dg + view, :],
                        w_view[:, :, dg * P:(dg + 1) * P])
            nc.scalar.dma_start(wv_sb[:], wv_v[:])
            nc.gpsimd.dma_start(wp_sb[:], wp_v[:])

            # ---------------- Phase A: projections ----------------
            # dg-major so phase B's (j, pr) work unblocks early
            for dg in range(NPAIR):
                for view, dstT in ((0, qT), (1, kT)):
                    for jj in range(NT):
                        ps = psA.tile([P, TT], f32, name="ps_qk", tag="psA")
                        for co in range(8):
                            nc.tensor.matmul(
                                ps[:], wqk_sb[:, co, 2 * dg + view, :],
                                xT_sb[:, co, jj * TT:(jj + 1) * TT],
                                start=(co == 0), stop=(co == 7))
                        nc.vector.tensor_copy(
                            out=dstT[:, dg, jj * TT:(jj + 1) * TT], in_=ps[:])
                # v for t128 in this dg's quarter
                for tq in range(4):
                    t128 = 4 * dg + tq
                    ps = psA.tile([P, G], f32, name="ps_v", tag="psA")
                    for co in range(8):
                        nc.tensor.matmul(
                            ps[:], xT_sb[:, co, t128 * P:(t128 + 1) * P],
                            wv_sb[:, co, :],
                            start=(co == 0), stop=(co == 7))
                    nc.vector.tensor_copy(
                        out=v_sb[:, t128, :, 0:64],
                        in_=ps.rearrange("p (h d) -> p h d", h=8))

            # ---------------- Phase B: attention ----------------
            for j in range(NT):
                ns = 4 * (j + 1)  # s tiles of 128 in causal prefix
                # diagonal s-tiles first so the GPSIMD mask overlaps the
                # remaining QK/exp stream and PV can start early
                so_order = list(range(4 * j, 4 * j + 4)) + list(range(4 * j))
                for pr in range(NPAIR):
                    # pair tile: [s-part, s-tile, head-in-pair, t]
                    expp = epool.tile(
                        [P, NT128, 2, TT], bf16, name="expp", tag="expp")
                    for si, so in enumerate(so_order):
                        ps_s = psS.tile([P, 2, TT], f32, name="ps_s", tag="psS")
                        for hi in range(2):
                            hp = 64 * hi
                            nc.tensor.matmul(
                                ps_s[:, hi, :],
                                kT[hp:hp + 64, pr, so * P:(so + 1) * P],
                                qT[hp:hp + 64, pr, j * TT:(j + 1) * TT],
                                start=True, stop=True)
                        nc.scalar.activation(
                            out=expp[:, so, :, :],
                            in_=ps_s[:],
                            func=mybir.ActivationFunctionType.Exp,
                            scale=0.125)
                        if si == 3:
                            # causal mask on diagonal 4 s-tiles (s > t -> 0)
                            for hi in range(2):
                                nc.gpsimd.affine_select(
                                    out=expp[:, 4 * j:4 * j + 4, hi, :],
                                    in_=expp[:, 4 * j:4 * j + 4, hi, :],
                                    pattern=[[-P, 4], [1, TT]],
                                    compare_op=mybir.AluOpType.is_ge,
                                    fill=0.0,
                                    base=0,
                                    channel_multiplier=-1)
                    # P^T @ [v | 1] accumulating out^T (65 rows) per head
                    for hi in range(2):
                        h = 2 * pr + hi
                        hp = 64 * hi
                        ps_o = psA.tile([P, TT], f32, name="ps_o", tag="psA")
                        for si, so in enumerate(so_order):
                            nc.tensor.matmul(
                                ps_o[0:65, :],
                                v_sb[:, so, h, 0:65],
                                expp[:, so, hi, :],
                                start=(si == 0), stop=(si == ns - 1))
                        nc.vector.tensor_copy(
                            out=ylocT[hp:hp + 64, pr, j * TT:(j + 1) * TT],
                            in_=ps_o[0:64, :])
                        nc.vector.tensor_copy(
                            out=lq[32 * j:32 * j + 1, h, :],
                            in_=ps_o[64:65, :])

            # ---------------- Phase C: normalize ----------------
            # hc-major so phase D's first half unblocks while hc=1 runs
            for hc in range(2):
                for jj in range(2):
                    j = 2 * hc + jj
                    for pr in range(NPAIR):
                        nc.sync.dma_start(
                            out=l8[32 * pr:32 * pr + 2, j, :],
                            in_=lq[32 * j:32 * j + 1, 2 * pr:2 * pr + 2, :])
                nc.vector.reciprocal(
                    out=l8[:, 2 * hc:2 * hc + 2, :],
                    in_=l8[:, 2 * hc:2 * hc + 2, :])
                # bcast[m, t] = sel[0, m]*recip_h0[t] + sel[1, m]*recip_h1[t]
                for pr in range(NPAIR):
                    rf = rfpool.tile([2, 2, TT], f32, name="rf", tag="rf")
                    nc.sync.dma_start(
                        out=rf[:],
                        in_=l8[32 * pr:32 * pr + 2, 2 * hc:2 * hc + 2, :])
                    ps_b = psS.tile([P, 2, TT], f32, name="ps_b", tag="psS")
                    for u in range(2):
                        nc.tensor.matmul(
                            ps_b[:, u, :], sel_sb[:], rf[:, u, :],
                            start=True, stop=True)
                    bc = bpool.tile([P, 2, TT], f32, name="bc", tag="bc")
                    nc.vector.tensor_copy(out=bc[:], in_=ps_b[:])
                    yv = ylocT[:, pr, hc * 1024:(hc + 1) * 1024]
                    nc.vector.tensor_tensor(
                        out=yv.rearrange("p (a b) -> p a b", a=2),
                        in0=yv.rearrange("p (a b) -> p a b", a=2),
                        in1=bc[:],
                        op=mybir.AluOpType.mult)

            # ---------------- Phase D: output projection ----------------
            for t128 in range(NT128):
                for cn in range(2):
                    ps_y = psA.tile([P, TT], f32, name="ps_y", tag="psA")
                    for uo in range(4):
                        nc.tensor.matmul(
                            ps_y[:],
                            ylocT[:, uo, t128 * P:(t128 + 1) * P],
                            wp_sb[:, uo, cn * TT:(cn + 1) * TT],
                            start=(uo == 0), stop=(uo == 3))
                    yst = ypool.tile([P, TT], f32, name="yst", tag="yst")
                    nc.vector.tensor_copy(out=yst[:], in_=ps_y[:])
                    nc.sync.dma_start(
                        out=y_v[:, t128, cn * TT:(cn + 1) * TT],
                        in_=yst[:])

    nc.finalize()
    return nc


def _get_nc():
    if "nc" not in _CACHE:
        _CACHE["nc"] = _build_nc()
    return _CACHE["nc"]


def _sel_array():
    sel = np.zeros((2, P), np.float32)
    sel[0, 0:64] = 1.0
    sel[1, 64:128] = 1.0
    return sel


def shard_inputs(x, Wq, Wk, Wv, Wp):
    """Build the 8 per-core input maps."""
    import ml_dtypes
    bf = ml_dtypes.bfloat16
    x = np.asarray(x, np.float32)
    Wq, Wk, Wv, Wp = (np.asarray(w, np.float32) for w in (Wq, Wk, Wv, Wp))
    in_maps = []
    for c in range(8):
        b, g = c // 2, c % 2
        sl = slice(g * G, (g + 1) * G)
        in_maps.append({
            "xt": np.ascontiguousarray(x[b].T).astype(bf),
            "wq": np.ascontiguousarray(Wq[:, sl]).astype(bf),
            "wk": np.ascontiguousarray(Wk[:, sl]).astype(bf),
            "wv": np.ascontiguousarray(Wv[:, sl]).astype(bf),
            "wp": np.ascontiguousarray(Wp[sl, :]).astype(bf),
            "sel": _sel_array(),
        })
    return in_maps


def unshard_outputs(results):
    """results: list of 8 dicts with 'y' [T, C] partials -> [B, T, C]."""
    out = np.empty((B, T, C), np.float32)
    for b in range(B):
        out[b] = results[2 * b]["y"] + results[2 * b + 1]["y"]
    return out


def kernel(**inputs):
    from concourse import bass_utils
    nc = _get_nc()
    in_maps = shard_inputs(**inputs)
    res = bass_utils.run_bass_kernel_spmd(nc, in_maps, core_ids=list(range(8)))
    return unshard_outputs(res.results)


# revision 18
# speedup vs baseline: 7343.6865x; 7343.6865x over previous
"""Causal self-attention Bass/Tile kernel for Trainium2, 8 NeuronCores.

Problem: B=4, T=2048, C=1024, NH=16, HD=64.
  q/k/v = x @ W{q,k,v}; att = softmax(causal(q k^T / 8)); y = (att v) @ Wp

Sharding (8 cores): batch (4-way) x head-group (2-way tensor parallel).
Core c handles batch b=c//2 and global heads g*8..g*8+7 where g=c%2.
Each core computes a partial projection y_part = y_heads_local @ Wp[rows]
and the host unshards by summing the two partial outputs per batch.

Per-core kernel (all T=2048 tokens, 8 heads, head_dim 64), bf16 matmuls
with fp32 PSUM accumulation and fp32 softmax:
  Phase A: x^T, Wq, Wk, Wv resident in SBUF (bf16); qT/kT = (x W)^T
           stored [d, t], v stored [t, d] augmented with a ones column so
           P^T@[V|1] also yields the softmax denominator l in PSUM row 64.
  Phase B: per query tile j / head pair: transposed score tiles
           S^T [s:128, t:512] on PE with the two heads of the pair on
           disjoint PE row halves (concurrent sub-array execution),
           exp(S/8) on ACT (PSUM->SBUF bf16), causal mask via GPSIMD
           affine_select (fill 0 post-exp), P^T@[V|1] accumulating
           unnormalized out^T [65, t] per head in PSUM.
  Phase C: l -> 1/l (DVE reciprocal), pair-broadcast over 128 head dims
           via a K=2 fp32 selector matmul, normalize out^T (DVE multiply).
  Phase D: y_part[t, c] = sum_u ylocT[u, t] * Wp[u, c] on PE (bf16).
"""

import numpy as np

B, T, C, NH, HD = 4, 2048, 1024, 16, 64
G = 512          # local head dims per core (8 heads x 64)
P = 128
NT = 4           # t tiles of 512
NT128 = 16       # t tiles of 128
NPAIR = 4        # local head pairs
TT = 512

_CACHE = {}


def _build_nc():
    import concourse.tile as tile
    from concourse import bacc, mybir

    f32 = mybir.dt.float32
    bf16 = mybir.dt.bfloat16

    nc = bacc.Bacc("TRN2", target_bir_lowering=False, debug=False)

    xT = nc.dram_tensor("xt", [C, T], bf16, kind="ExternalInput")
    wq = nc.dram_tensor("wq", [C, G], bf16, kind="ExternalInput")
    wk = nc.dram_tensor("wk", [C, G], bf16, kind="ExternalInput")
    wv = nc.dram_tensor("wv", [C, G], bf16, kind="ExternalInput")
    wp = nc.dram_tensor("wp", [G, C], bf16, kind="ExternalInput")
    sel = nc.dram_tensor("sel", [2, P], f32, kind="ExternalInput")
    y = nc.dram_tensor("y", [T, C], f32, kind="ExternalOutput")

    xT_v = xT.rearrange("(co p) t -> p co t", p=P)      # [128, 8, 2048]
    wq_v = wq.rearrange("(co p) g -> p co g", p=P)      # [128, 8, 512]
    wk_v = wk.rearrange("(co p) g -> p co g", p=P)
    wv_v = wv.rearrange("(co p) g -> p co g", p=P)
    wp_v = wp.rearrange("(uo p) c -> p uo c", p=P)      # [128, 4, 1024]
    y_v = y.rearrange("(to p) c -> p to c", p=P)        # [128, 16, 1024]

    with tile.TileContext(nc) as tc:
        with (
            tc.tile_pool(name="singles", bufs=1) as singles,
            tc.tile_pool(name="expst", bufs=2) as epool,
            tc.tile_pool(name="bcast", bufs=1) as bpool,
            tc.tile_pool(name="rf", bufs=2) as rfpool,
            tc.tile_pool(name="ystage", bufs=3) as ypool,
            tc.tile_pool(name="psA", bufs=4, space="PSUM") as psA,
            tc.tile_pool(name="psS", bufs=2, space="PSUM") as psS,
        ):
            # persistent tensors
            xT_sb = singles.tile([P, 8, T], bf16, name="xT_sb", tag="xT_sb")
            # wqk_sb[:, co, 2*dg+view, :]: lhsT tiles for q (view 0), k (view 1)
            wqk_sb = singles.tile([P, 8, 8, P], bf16, name="wqk_sb", tag="wqk_sb")
            wv_sb = singles.tile([P, 8, G], bf16, name="wv_sb", tag="wv_sb")
            wp_sb = singles.tile([P, NPAIR, C], bf16, name="wp_sb", tag="wp_sb")
            qT = singles.tile([P, NPAIR, T], bf16, name="qT", tag="qT")
            kT = singles.tile([P, NPAIR, T], bf16, name="kT", tag="kT")
            v_sb = singles.tile([P, NT128, 8, 66], bf16, name="v_sb", tag="v_sb")
            ylocT = singles.tile([P, NPAIR, T], bf16, name="ylocT", tag="ylocT")
            # l for (h, j) lives at partition 32*j, free slot h (DVE copies
            # out of PSUM row 64 may only target partitions 0/32/64/96);
            # l8 holds pair pr at partitions {32pr, 32pr+1}
            lq = singles.tile([P, 8, TT], f32, name="lq", tag="lq")
            l8 = singles.tile([P, NT, TT], f32, name="l8", tag="l8")
            sel_sb = singles.tile([2, P], f32, name="sel_sb", tag="sel_sb")

            nc.vector.memset(v_sb[:, :, :, 64:65], 1.0)
            nc.vector.memset(l8[:], 1.0)
            nc.gpsimd.dma_start(sel_sb[:], sel[:])
            # load order: dg0 weights first so PE starts ~immediately, then
            # x chunks; remaining weights ride other engines' DMA queues.
            nc.sync.dma_start(wqk_sb[:, :, 0, :], wq_v[:, :, 0:P])
            nc.scalar.dma_start(wqk_sb[:, :, 1, :], wk_v[:, :, 0:P])
            for th in range(2):
                for co in range(8):
                    nc.sync.dma_start(
                        xT_sb[:, co, th * 1024:(th + 1) * 1024],
                        xT_v[:, co, th * 1024:(th + 1) * 1024])
            for dg in range(1, NPAIR):
                for view, w_view in ((0, wq_v), (1, wk_v)):
                    nc.scalar.dma_start(
                        wqk_sb[:, :, 2 * dg + view, :],
                        w_view[:, :, dg * P:(dg + 1) * P])
            nc.scalar.dma_start(wv_sb[:], wv_v[:])
            nc.gpsimd.dma_start(wp_sb[:], wp_v[:])

            # ----- Phases A (projections) and B (attention), interleaved -----
            # A's PE-dense blocks are emitted between B's ACT-paced blocks so
            # the scheduler can fill PE idle time while ACT streams exps.
            def emit_A(dg):
                for view, dstT in ((0, qT), (1, kT)):
                    for jj in range(NT):
                        ps = psA.tile([P, TT], f32, name="ps_qk", tag="psA")
                        for co in range(8):
                            nc.tensor.matmul(
                                ps[:], wqk_sb[:, co, 2 * dg + view, :],
                                xT_sb[:, co, jj * TT:(jj + 1) * TT],
                                start=(co == 0), stop=(co == 7))
                        nc.vector.tensor_copy(
                            out=dstT[:, dg, jj * TT:(jj + 1) * TT], in_=ps[:])
                for tq in range(4):
                    t128 = 4 * dg + tq
                    ps = psA.tile([P, G], f32, name="ps_v", tag="psA")
                    for co in range(8):
                        nc.tensor.matmul(
                            ps[:], xT_sb[:, co, t128 * P:(t128 + 1) * P],
                            wv_sb[:, co, :],
                            start=(co == 0), stop=(co == 7))
                    nc.vector.tensor_copy(
                        out=v_sb[:, t128, :, 0:64],
                        in_=ps.rearrange("p (h d) -> p h d", h=8))

            def emit_B(j, pr):
                ns = 4 * (j + 1)  # s tiles of 128 in causal prefix
                # diagonal s-tiles first so the GPSIMD mask overlaps the
                # remaining QK/exp stream and PV can start early
                so_order = list(range(4 * j, 4 * j + 4)) + list(range(4 * j))
                expp_lo = epool.tile(
                    [P, 8, 2, TT], bf16, name="expp_lo", tag="expp")
                expp_hi = expp_lo if ns <= 8 else epool.tile(
                    [P, 8, 2, TT], bf16, name="expp_hi", tag="expp")

                def eslc(so, hi_, _lo=expp_lo, _hi=expp_hi):
                    t = _lo if so < 8 else _hi
                    return t[:, so % 8, hi_, :]

                def eslc4(lo4, hi_, _lo=expp_lo, _hi=expp_hi):
                    t = _lo if lo4 < 8 else _hi
                    return t[:, lo4 % 8:lo4 % 8 + 4, hi_, :]
                for si, so in enumerate(so_order):
                    ps_s = psS.tile([P, 2, TT], f32, name="ps_s", tag="psS")
                    for hi in range(2):
                        hp = 64 * hi
                        nc.tensor.matmul(
                            ps_s[:, hi, :],
                            kT[hp:hp + 64, pr, so * P:(so + 1) * P],
                            qT[hp:hp + 64, pr, j * TT:(j + 1) * TT],
                            start=True, stop=True)
                    nc.scalar.activation(
                        out=(expp_lo if so < 8 else expp_hi)[:, so % 8, :, :],
                        in_=ps_s[:],
                        func=mybir.ActivationFunctionType.Exp,
                        scale=0.125)
                    if si == 3:
                        # causal mask on diagonal 4 s-tiles (s > t -> 0)
                        for hi in range(2):
                            nc.gpsimd.affine_select(
                                out=eslc4(4 * j, hi),
                                in_=eslc4(4 * j, hi),
                                pattern=[[-P, 4], [1, TT]],
                                compare_op=mybir.AluOpType.is_ge,
                                fill=0.0,
                                base=0,
                                channel_multiplier=-1)
                # P^T @ [v | 1] accumulating out^T (65 rows) per head
                for hi in range(2):
                    h = 2 * pr + hi
                    hp = 64 * hi
                    ps_o = psA.tile([P, TT], f32, name="ps_o", tag="psA")
                    for si, so in enumerate(so_order):
                        nc.tensor.matmul(
                            ps_o[0:65, :],
                            v_sb[:, so, h, 0:65],
                            eslc(so, hi),
                            start=(si == 0), stop=(si == ns - 1))
                    nc.vector.tensor_copy(
                        out=ylocT[hp:hp + 64, pr, j * TT:(j + 1) * TT],
                        in_=ps_o[0:64, :])
                    nc.vector.tensor_copy(
                        out=lq[32 * j:32 * j + 1, h, :],
                        in_=ps_o[64:65, :])

            for dg in range(NPAIR):
                emit_A(dg)
            for j in range(NT):
                for pr in range(NPAIR):
                    emit_B(j, pr)

            # ---------------- Phase C: normalize ----------------
            # hc-major so phase D's first half unblocks while hc=1 runs
            for hc in range(2):
                for jj in range(2):
                    j = 2 * hc + jj
                    for pr in range(NPAIR):
                        nc.sync.dma_start(
                            out=l8[32 * pr:32 * pr + 2, j, :],
                            in_=lq[32 * j:32 * j + 1, 2 * pr:2 * pr + 2, :])
                nc.vector.reciprocal(
                    out=l8[:, 2 * hc:2 * hc + 2, :],
                    in_=l8[:, 2 * hc:2 * hc + 2, :])
                # bcast[m, t] = sel[0, m]*recip_h0[t] + sel[1, m]*recip_h1[t]
                for pr in range(NPAIR):
                    rf = rfpool.tile([2, 2, TT], f32, name="rf", tag="rf")
                    nc.sync.dma_start(
                        out=rf[:],
                        in_=l8[32 * pr:32 * pr + 2, 2 * hc:2 * hc + 2, :])
                    ps_b = psS.tile([P, 2, TT], f32, name="ps_b", tag="psS")
                    for u in range(2):
                        nc.tensor.matmul(
                            ps_b[:, u, :], sel_sb[:], rf[:, u, :],
                            start=True, stop=True)
                    bc = bpool.tile([P, 2, TT], f32, name="bc", tag="bc")
                    nc.vector.tensor_copy(out=bc[:], in_=ps_b[:])
                    yv = ylocT[:, pr, hc * 1024:(hc + 1) * 1024]
                    nc.vector.tensor_tensor(
                        out=yv.rearrange("p (a b) -> p a b", a=2),
                        in0=yv.rearrange("p (a b) -> p a b", a=2),
                        in1=bc[:],
                        op=mybir.AluOpType.mult)

            # ---------------- Phase D: output projection ----------------
            for t128 in range(NT128):
                for cn in range(2):
                    ps_y = psA.tile([P, TT], f32, name="ps_y", tag="psA")
                    for uo in range(4):
                        nc.tensor.matmul(
                            ps_y[:],
                            ylocT[:, uo, t128 * P:(t128 + 1) * P],
                            wp_sb[:, uo, cn * TT:(cn + 1) * TT],
                            start=(uo == 0), stop=(uo == 3))
                    yst = ypool.tile([P, TT], f32, name="yst", tag="yst")
                    nc.vector.tensor_copy(out=yst[:], in_=ps_y[:])
                    nc.sync.dma_start(
                        out=y_v[:, t128, cn * TT:(cn + 1) * TT],
                        in_=yst[:])

    nc.finalize()
    return nc


def _get_nc():
    if "nc" not in _CACHE:
        _CACHE["nc"] = _build_nc()
    return _CACHE["nc"]


def _sel_array():
    sel = np.zeros((2, P), np.float32)
    sel[0, 0:64] = 1.0
    sel[1, 64:128] = 1.0
    return sel


def shard_inputs(x, Wq, Wk, Wv, Wp):
    """Build the 8 per-core input maps."""
    import ml_dtypes
    bf = ml_dtypes.bfloat16
    x = np.asarray(x, np.float32)
    Wq, Wk, Wv, Wp = (np.asarray(w, np.float32) for w in (Wq, Wk, Wv, Wp))
    in_maps = []
    for c in range(8):
        b, g = c // 2, c % 2
        sl = slice(g * G, (g + 1) * G)
        in_maps.append({
            "xt": np.ascontiguousarray(x[b].T).astype(bf),
            "wq": np.ascontiguousarray(Wq[:, sl]).astype(bf),
            "wk": np.ascontiguousarray(Wk[:, sl]).astype(bf),
            "wv": np.ascontiguousarray(Wv[:, sl]).astype(bf),
            "wp": np.ascontiguousarray(Wp[sl, :]).astype(bf),
            "sel": _sel_array(),
        })
    return in_maps


def unshard_outputs(results):
    """results: list of 8 dicts with 'y' [T, C] partials -> [B, T, C]."""
    out = np.empty((B, T, C), np.float32)
    for b in range(B):
        out[b] = results[2 * b]["y"] + results[2 * b + 1]["y"]
    return out


def kernel(**inputs):
    from concourse import bass_utils
    nc = _get_nc()
    in_maps = shard_inputs(**inputs)
    res = bass_utils.run_bass_kernel_spmd(nc, in_maps, core_ids=list(range(8)))
    return unshard_outputs(res.results)
